# revision 6
# baseline (speedup 1.0000x reference)
"""CARAFE content-aware upsampling kernel for Trainium2 (Bass/Tile), 8 NeuronCores.

Problem (hardcoded): features [4, 256, 64, 64] f32, masks [4, 25, 128, 128] f32,
K=5, G=1, S=2 -> output [4, 256, 128, 128] f32.

Strategy
--------
Sharding: 8 cores = (batch n in 0..3) x (output-row half yh in 0..1); each core
computes out[n, :, yh*64:(yh+1)*64, :] for ALL 256 channels.

Compute mapping: each output block of (4 row-pairs x 16 columns) = 128 output
positions depends on an 8-row x 12-col window of the padded input feature map.
Flattening that window gives a 96-long contraction axis (k = wpw*8 + hpw)
that covers ALL 25 CARAFE taps in a single matmul:

  out[c, pos] = sum_k ftr[k, c] * bnd[k, pos],   k = (wpw, hpw) in 12 x 8

ftr is the host-replicated feature window per block (bf16); bnd is a
host-built banded mask operand (bf16): bnd[(wpw,hpw), (p4,py,xl)] =
mask[kr*5+dw, y, x] with kr = hpw-p4, dw = wpw-xl//2 when both fall in [0,5),
else zero. One matmul per (block, channel-half): 128 matmuls of 128 moving
columns each per core (~16k PE cycles), and the PSUM output lands directly in
[c, y-major] layout so no transpose is needed anywhere.

Edge columns: for xblk 0 the contraction rows wpw in {0,1} hit zero-padded
feature columns (wp in {0,1}); for xblk 7 the rows wpw in {10,11} do. With the
wpw-major row order those rows are contiguous ([0,16) resp. [80,96)), so edge
tiles ship only 80 contraction rows and the matmul contracts the sub-range.

Dataflow: ftr and bnd are packed per row-group chunk into one DRAM tensor
(full-K block + two edge blocks) and streamed with 3 DMAs per chunk; per
row-group a [128, 2048] PSUM tile (4 banks) collects 16 independent matmuls
(start=stop=True each, disjoint columns); DVE and ACT each cast one
channel-half to bf16 into a shared staging tile (reordering to y-major), and
one DMA per row-group writes [256ch x 8row x 128col] to DRAM. The host
upcasts to f32.
"""

import sys

sys.path.insert(0, "/opt/trn_rl_repo")

import numpy as np
import ml_dtypes

import concourse.bacc as bacc
import concourse.mybir as mybir
from concourse import tile
from concourse import bass_utils

N, C, H, W = 4, 256, 64, 64
KK = 5
HO, WO = 128, 128
NCORES = 8

HPL = 36          # padded input rows per core (32 pairs + 4 tap overlap)
WP = 68           # padded input cols
NHG = 8           # row-groups per core (4 row-pairs each)
NXB = 8           # col-blocks per core (16 output cols each)
KDIM = 96         # contraction: 12 wp x 8 hp
KE = 80           # contraction rows for edge col-blocks (xblk 0 and 7)
CHUNKS = (1, 1, 2, 2, 2)   # hgrps per input DMA chunk

# per-hgrp fbd column layout: mid block (xblk 1..6, 96 rows), then edge
# blocks (xblk 0 rows [16:96], xblk 7 rows [0:80])
MIDW = 6 * (256 + 128)     # 2304 cols, 96 rows
EDGW = 256 + 128           # 384 cols per edge block, 80 rows
HGW = MIDW + 2 * EDGW      # 3072 cols per hgrp (at mixed row counts)

BF16 = ml_dtypes.bfloat16


def _host_prep(features: np.ndarray, masks: np.ndarray):
    """Per-core packed (ftr || bnd) chunk operands.

    Returns fbm [96, NHG*MIDW] (mid blocks) and fbe [80, NHG*2*EDGW]
    (edge blocks, xblk0 then xblk7 per hgrp), both bf16.
    """
    featT = features.transpose(0, 2, 3, 1)  # [N, H, W, C]
    fbms, fbes = [], []
    for i in range(NCORES):
        n, yh = divmod(i, 2)
        # padded transposed features: feat_pad[hp, wp, c] = features[n, c, yh*32+hp-2, wp-2]
        feat_pad = np.zeros((HPL, WP, C), np.float32)
        r0 = yh * 32 - 2
        lo, hi = max(0, -r0), min(HPL, H - r0)
        feat_pad[lo:hi, 2:2 + W, :] = featT[n, r0 + lo:r0 + hi]

        # ftr[k=(wpw*8+hpw), t=(hgrp, xblk), c]
        s_hp, s_wp, s_c = feat_pad.strides
        ftr = np.lib.stride_tricks.as_strided(
            feat_pad,
            shape=(12, 8, NHG, NXB, C),
            strides=(s_wp, s_hp, 4 * s_hp, 8 * s_wp, s_c),
        ).reshape(KDIM, NHG, NXB, C)

        # bnd[k=(wpw,hpw), (hgrp, xblk), (p4, py, xl)]
        ml = masks[n, :, yh * 64:(yh + 1) * 64, :]  # [25, 64, 128]
        bnd = np.zeros((12, 8, NHG, NXB, 4, 2, 16), np.float32)
        s = bnd.strides
        for kr in range(KK):
            for dw in range(KK):
                # dest dims (p4, hgrp, xblk, xw, py, q):
                #   bnd[dw+xw, p4+kr, hgrp, xblk, p4, py, 2*xw+q]
                dv = np.lib.stride_tricks.as_strided(
                    bnd[dw, kr],
                    shape=(4, NHG, NXB, 8, 2, 2),
                    strides=(s[1] + s[4], s[2], s[3], s[0] + 2 * s[6], s[5], s[6]),
                )
                sv = ml[kr * KK + dw].reshape(NHG, 4, 2, NXB, 8, 2)
                dv[...] = sv.transpose(1, 0, 3, 4, 2, 5)
        bnd = bnd.reshape(KDIM, NHG, NXB, 128)

        fbm = np.empty((KDIM, NHG, MIDW), np.float32)
        fbm[:, :, :6 * C] = ftr[:, :, 1:7].reshape(KDIM, NHG, 6 * C)
        fbm[:, :, 6 * C:] = bnd[:, :, 1:7].reshape(KDIM, NHG, 6 * 128)
        fbe = np.empty((KE, NHG, 2, EDGW), np.float32)
        fbe[:, :, 0, :C] = ftr[16:, :, 0]
        fbe[:, :, 0, C:] = bnd[16:, :, 0]
        fbe[:, :, 1, :C] = ftr[:KE, :, 7]
        fbe[:, :, 1, C:] = bnd[:KE, :, 7]
        fbms.append(fbm.reshape(KDIM, NHG * MIDW).astype(BF16))
        fbes.append(fbe.reshape(KE, NHG * 2 * EDGW).astype(BF16))
    return fbms, fbes


_NC_CACHE = []


def _build_nc():
    """Build + compile the single-core Tile program (same for all 8 cores)."""
    if _NC_CACHE:
        return _NC_CACHE[0]

    nc = bacc.Bacc("TRN2", target_bir_lowering=False, debug=False)
    fbm = nc.dram_tensor(
        "fbm", [KDIM, NHG * MIDW], mybir.dt.bfloat16, kind="ExternalInput"
    ).ap()
    fbe = nc.dram_tensor(
        "fbe", [KE, NHG * 2 * EDGW], mybir.dt.bfloat16, kind="ExternalInput"
    ).ap()
    out = nc.dram_tensor(
        "out", [C, 64 * 128], mybir.dt.bfloat16, kind="ExternalOutput"
    ).ap()
    # out view [ch, c, hgrp, f=1024]
    ov = out.rearrange("(ch c) (hgrp f) -> ch c hgrp f", ch=2, hgrp=NHG)

    with tile.TileContext(nc) as tc:
        with (
            tc.tile_pool(name="fbp", bufs=2 * len(CHUNKS)) as fbp,
            tc.tile_pool(name="stp", bufs=8) as stp,
            tc.tile_pool(name="pp", bufs=2, space="PSUM") as pp,
        ):
            chunk_of = []   # per hgrp: (mid tile, edge tile, local hgrp idx)
            moff = eoff = 0
            for ci, g in enumerate(CHUNKS):
                tm = fbp.tile([KDIM, g * MIDW], mybir.dt.bfloat16,
                              name="fbm", tag="fbm")
                te = fbp.tile([KE, g * 2 * EDGW], mybir.dt.bfloat16,
                              name="fbe", tag="fbe")
                eng = nc.gpsimd if ci == 0 else nc.sync
                eng.dma_start(tm[:], fbm[:, moff:moff + g * MIDW])
                nc.sync.dma_start(te[:], fbe[:, eoff:eoff + g * 2 * EDGW])
                moff += g * MIDW
                eoff += g * 2 * EDGW
                for hh in range(g):
                    chunk_of.append((tm, te, hh))

            for hgrp in range(NHG):
                tm, te, hh = chunk_of[hgrp]
                ps = pp.tile([128, 2048], mybir.dt.float32, name="ps", tag="ps")
                for xblk in range(NXB):
                    if xblk in (0, 7):
                        e = 0 if xblk == 0 else 1
                        base = (hh * 2 + e) * EDGW
                        rhs = te[:, base + C: base + C + 128]
                        lhs = [te[:, base + ch * 128: base + (ch + 1) * 128]
                               for ch in range(2)]
                    else:
                        base = hh * MIDW + (xblk - 1) * C
                        bb = hh * MIDW + 6 * C + (xblk - 1) * 128
                        rhs = tm[:, bb: bb + 128]
                        lhs = [tm[:, base + ch * 128: base + (ch + 1) * 128]
                               for ch in range(2)]
                    for ch in range(2):
                        nc.tensor.matmul(
                            ps[:, (xblk * 2 + ch) * 128: (xblk * 2 + ch + 1) * 128],
                            lhs[ch],
                            rhs,
                            start=True,
                            stop=True,
                        )
                st = stp.tile([128, 2048], mybir.dt.bfloat16, name="st", tag="st")
                # psum cols (xblk, ch, p4, py, xl) -> staging cols (ch, p4, py, xblk, xl)
                sv = ps.rearrange(
                    "c (xblk ch p4 py xl) -> c ch xblk p4 py xl",
                    xblk=8, ch=2, p4=4, py=2,
                )
                dv = st.rearrange(
                    "c (ch p4 py xblk xl) -> c ch xblk p4 py xl",
                    ch=2, p4=4, py=2, xblk=8,
                )
                nc.vector.tensor_copy(dv[:, 0], sv[:, 0])
                nc.scalar.copy(dv[:, 1], sv[:, 1])
                # staging [c, (ch f)] -> out[ch*128+c, hgrp*1024 + f]
                sov = st.rearrange("c (ch f) -> c ch f", ch=2)
                nc.sync.dma_start(ov[:, :, hgrp, :].rearrange("ch c f -> c ch f"), sov)

    nc.compile()
    _NC_CACHE.append(nc)
    return nc


def kernel(features: np.ndarray, masks: np.ndarray) -> np.ndarray:
    features = np.ascontiguousarray(features, dtype=np.float32)
    masks = np.ascontiguousarray(masks, dtype=np.float32)
    fbms, fbes = _host_prep(features, masks)

    nc = _build_nc()
    in_maps = [{"fbm": fbms[i], "fbe": fbes[i]} for i in range(NCORES)]

    res = bass_utils.run_bass_kernel_spmd(nc, in_maps, list(range(NCORES)))

    out = np.empty((N, C, HO, WO), np.float32)
    for i in range(NCORES):
        n, yh = divmod(i, 2)
        out[n, :, yh * 64:(yh + 1) * 64, :] = (
            res.results[i]["out"].astype(np.float32).reshape(C, 64, 128)
        )
    return out
